# revision 62
# baseline (speedup 1.0000x reference)
"""Bass/Trainium2 kernel for the DisentangleLoss (NT-Xent style contrastive loss).

Math (matches the reference):
    sn = s / max(||s||, eps) * sqrt(1/TEMP)      row-normalized embeddings
    sim = sn @ sn.T                              [K, K] similarity logits
    positives of row i: columns j != i with |i-j| = 1024*m   (8 per row)
    negatives of row i: everything else except the diagonal  (K-9 per row)
    loss = mean over (row, positive) of  logaddexp(p, lse(negatives)) - p

Device strategy (8 NeuronCores, SPMD) -- "circulant band" symmetric scheme:
  sim is symmetric, so only half the exp(sim) evaluations are needed.  Core c
  gets a row-rolled copy of s (np.roll by -1152*c) and computes, for its 9
  local row tiles Tb (128 rows each), the sim blocks (Tb, Tb+d) for band
  offsets d = 0..36.  Globally every unordered tile pair {A, B} is covered
  exactly once (d in 1..35), the diagonal tiles once (d=0), and the d=36 pairs
  exactly twice (so those skip their column sums).  Each core:
    * loads rows 0..6143 of its rolled s (tiles 0..44 are all it needs),
      normalizes (squares on Pool / fused sum-of-squares on DVE, rsqrt via
      ln/exp on ACT, per-tile scale+bf16-cast muls) and PE-transposes tiles
      0..44 into snT.  Emission order is tuned so the strict per-engine FIFOs
      never head-of-line block: norm phases fire as loads land, transpose
      phases a few units later, and each unit's colsum/diag consumption is
      deferred two units behind its matmul+exp production.
    * for each (row tile Tb, d-panel of 12): one bf16 matmul row [128, 1536]
      into PSUM, then exp via ACT (accum_out = row-sum partial; ex to SBUF
      bf16) or, for OFFLOAD units and the d=36 blocks, DVE Schraudolph
      bit-trick exp + fused f32 row-sum (balances the two PSUM readers).
    * column sums (the mirrored half of the symmetric matrix) via per-block
      "ones mat-vec": matmul(out=[128,1], lhsT=ex_block, rhs=ones) into a
      per-unit PSUM tile (PSUM accumulation across interleaved groups is NOT
      reliable on hw), added into an SBUF accumulator by one DVE op per unit.
    * positives live on the main diagonals of blocks d = 8,16,24,32;
      identity-mask multiply on Pool + fused row-sum accumulate on DVE
      (Pool supports no accumulators or scalar_tensor_tensor in the ISA).
    * self-similarity is exactly 2 after the sqrt(1/TEMP) scaling -> exp is
      the constant e^2, applied on the host.
  Each core outputs row-sum partials, column-sum partials and positive
  exponentials in one [128, 115] tensor; the host (O(K) numpy) assembles
  rowtot, per-row negsum and the final scalar loss.
"""

import math

import numpy as np

K = 9216
D = 128
BS = 1024
N = 9
TEMP = 0.5
NCORES = 8
R = K // NCORES          # 1152 rows per core
RT = R // 128            # 9 row tiles per core
NTILES = 45              # column tiles used per core (Tb + d <= 8 + 36)
NCHUNK = 6               # 1024-row load chunks (6144 rows >= 45*128)
KPOS = K * (N - 1)
E2 = math.exp(2.0)       # exp(self-similarity / TEMP)

# d-panels of 12 (ACT exp instructions are 1536 wide); d=36 handled separately
PANELS = [(0, 12), (12, 24), (24, 36)]
POS_DELTAS = (8, 16, 24, 32)
NP = len(PANELS) + 1     # rowscr columns per row tile (3 panels + d=36)

# Schraudolph bit-trick exp on DVE: exp(x) ~= bitcast_f32(int32(x*2^23/ln2+B));
# B fitted so the mean rel-err over the logit distribution is ~0 (max 3.3%
# per element, which averages out in the ~9k-term logsumexp).
SCHRAUDOLPH_S = 12102203.0
SCHRAUDOLPH_B = 1064951741.0
# (panel, Tb) units whose exp+rowsum runs on DVE instead of ACT
OFFLOAD = {(0, 7), (1, 2), (1, 5), (2, 1), (2, 4)}

# packed output layout
O_ROW = 0                # rowscr: NP per row tile        [0, 36)
O_COL = RT * NP          # colsums C=1..43                [36, 79)
O_EP = O_COL + 43        # exp(pos) per (Tb, m)           [79, 115)
O_W = O_EP + 36

_CACHE = {}


def _schedule():
    """Flattened main-loop schedule: list of ("unit", p, t) / ("d36", t) /
    ("chunk", ch).  All prologue chunks are emitted during panel 0 (each a
    little after its DMA load lands -- engine queues are strict FIFO, so a
    chunk emitted too early would head-of-line-block its engines); the d=36
    rowsum-only units are spread through panel 1."""
    norm_after = {(0, 1): 2, (0, 3): 3, (0, 5): 4, (0, 7): 5}
    trans_after = {(0, 4): 2, (0, 7): 3, (1, 1): 4, (1, 3): 5}
    sched = [("norm", -4), ("norm", -3), ("trans", -4), ("norm", -2),
             ("trans", -3), ("norm", -1), ("trans", -2), ("trans", -1)]
    for p, (lo, hi) in enumerate(PANELS):
        for t in range(RT):
            sched.append(("unit", p, t))
            if (p, t) in norm_after:
                sched.append(("norm", norm_after[(p, t)]))
            if (p, t) in trans_after:
                sched.append(("trans", trans_after[(p, t)]))
            # d36 units trail by 2 so their tiles (chunks 4-5) are ready
            if p == 1 and t >= 2:
                sched.append(("d36", t - 2))
            if p == 1 and t == 8:
                sched += [("d36", 7), ("d36", 8)]
    return sched


def _build():
    import concourse.bacc as bacc
    import concourse.tile as tile
    from concourse import mybir
    from concourse.masks import make_identity

    # Steer the ACT-table placement pass: every Exp/Ln in this kernel should
    # be served by the one set containing both ("natural_log_exp_and_others"),
    # otherwise the per-func first-match choice alternates tables and inserts
    # a ~2.7us ACT_TABLE_LOAD per switch.
    if not getattr(bacc, "_ant_act_tables_patched", False):
        _orig_get_tables = bacc.get_activation_tables

        def _patched_get_tables(arch):
            tables = dict(_orig_get_tables(arch))
            exp_ln = {mybir.ActivationFunctionType.Exp,
                      mybir.ActivationFunctionType.Ln}
            for name, funcs in tables.items():
                if name != "natural_log_exp_and_others" and \
                        exp_ln <= tables.get("natural_log_exp_and_others",
                                             set()):
                    tables[name] = funcs - exp_ln
            return tables

        bacc.get_activation_tables = _patched_get_tables
        bacc._ant_act_tables_patched = True

    f32 = mybir.dt.float32
    bf16 = mybir.dt.bfloat16
    i32 = mybir.dt.int32
    AF = mybir.ActivationFunctionType
    OP = mybir.AluOpType

    nc = bacc.Bacc("TRN2", target_bir_lowering=False, debug=False,
                   num_devices=NCORES)
    s_in = nc.dram_tensor("s", [NCHUNK * 1024, D], f32, kind="ExternalInput")
    y_out = nc.dram_tensor("out", [128, O_W], f32, kind="ExternalOutput")

    with tile.TileContext(nc) as tc:
        with (
            tc.tile_pool(name="big", bufs=1) as big,
            tc.tile_pool(name="small", bufs=1) as small,
            tc.tile_pool(name="ex", bufs=6) as ex_pool,
            tc.tile_pool(name="scr", bufs=4) as scr_pool,
            tc.tile_pool(name="pg", bufs=2, space="PSUM") as pg_pool,
            tc.tile_pool(name="tp", bufs=1, space="PSUM") as tp_pool,
            tc.tile_pool(name="cs", bufs=1, space="PSUM") as cs_pool,
        ):
            s_rows = big.tile([128, NCHUNK * 1024], f32)
            sn_rows = big.tile([128, NCHUNK * 1024], bf16)
            snT = big.tile([128, NTILES * 128], bf16)

            identf = small.tile([128, 128], f32)
            make_identity(nc, identf)
            ident_bf = small.tile([128, 128], bf16)
            nc.vector.tensor_copy(ident_bf[:], identf[:])
            ones_bf = small.tile([128, 1], bf16)
            nc.vector.memset(ones_bf, 1.0)
            ones_f = small.tile([128, 1], f32)
            nc.vector.memset(ones_f, 1.0)

            ss = small.tile([128, NCHUNK * 8], f32)   # per-row sum of squares
            lnss = small.tile([128, NCHUNK * 8], f32)
            sclr = small.tile([128, NCHUNK * 8], f32)
            scl = small.tile([128, NCHUNK * 8], f32)
            outbuf = small.tile([128, O_W], f32)      # packed outputs
            nc.vector.memset(outbuf[:, O_COL:O_COL + 43], 0.0)

            rt2 = math.sqrt(1.0 / TEMP)
            bias_t = small.tile([128, 1], f32)
            nc.vector.memset(bias_t, math.log(rt2))

            # tiny dummy exp so the ACT table loads while DMAs stream
            nc.scalar.activation(out=lnss[:, 0:1], in_=bias_t[:], func=AF.Exp)

            # ---- loads: chunks 0-1 as four 512-row half-loads (faster
            # startup; the first exp needs 12 tiles), chunks 2-5 whole.
            def emit_load(r0, r1, eng):
                src = s_in[r0:r1, :].rearrange("(t p) d -> p t d", p=128)
                dst = s_rows[:, r0:r1].rearrange("p (t d) -> p t d", d=128)
                eng.dma_start(out=dst, in_=src)

            emit_load(0, 512, nc.scalar)
            for h in range(1, 4):
                emit_load(h * 512, (h + 1) * 512, nc.sync)
            for ch in range(2, NCHUNK):
                emit_load(ch * 1024, (ch + 1) * 1024, nc.sync)

            def prologue_norm(tiles, fast):
                # fast=True puts the whole chain on DVE (kernel-start
                # critical path); otherwise squares and scale run on Pool
                # (its only ISA-legal roles) with row-sums+accum on DVE.
                tiles = [k for k in tiles if k < NTILES]
                if not tiles:
                    return
                w = len(tiles) * 128
                base = tiles[0] * 128
                if fast:
                    for k in tiles:
                        sl = slice(k * 128, (k + 1) * 128)
                        sc = scr_pool.tile([128, 128], f32, tag="ssq")
                        nc.vector.scalar_tensor_tensor(
                            out=sc, in0=s_rows[:, sl], scalar=1.0,
                            in1=s_rows[:, sl], op0=OP.mult, op1=OP.mult,
                            accum_out=ss[:, k:k + 1])
                else:
                    sq = scr_pool.tile([128, 1024], f32, tag="sq")
                    nc.gpsimd.tensor_tensor(
                        out=sq[:, :w], in0=s_rows[:, base:base + w],
                        in1=s_rows[:, base:base + w], op=OP.mult)
                    for i, k in enumerate(tiles):
                        dmy = scr_pool.tile([128, 128], f32, tag="ssqd")
                        nc.vector.tensor_scalar(
                            out=dmy, in0=sq[:, i * 128:(i + 1) * 128],
                            scalar1=1.0, scalar2=0.0, op0=OP.mult,
                            op1=OP.add, accum_out=ss[:, k:k + 1])
                gsl = slice(tiles[0], tiles[-1] + 1)
                nc.scalar.activation(out=lnss[:, gsl], in_=ss[:, gsl],
                                     func=AF.Ln)
                nc.scalar.activation(out=sclr[:, gsl], in_=lnss[:, gsl],
                                     func=AF.Exp, scale=-0.5, bias=bias_t[:])
                nc.vector.tensor_scalar_min(scl[:, gsl], sclr[:, gsl],
                                            rt2 * 1e8)
                # normalize+scale+cast to bf16 (per-tile scalar mul,
                # alternating Pool/DVE so neither serializes the chunk)
                for i, k in enumerate(tiles):
                    sl = slice(k * 128, (k + 1) * 128)
                    eng = nc.vector if (fast or i % 2) else nc.gpsimd
                    eng.tensor_scalar_mul(sn_rows[:, sl], s_rows[:, sl],
                                          scl[:, k:k + 1])

            def prologue_trans(tiles):
                # PE transpose into snT (via bf16 PSUM staging); emitted a
                # few units after the norm phase so the PE FIFO never waits
                # on the Pool-engine scale muls.
                tiles = [k for k in tiles if k < NTILES]
                if not tiles:
                    return
                w = len(tiles) * 128
                base = tiles[0] * 128
                pt = tp_pool.tile([128, 1024], bf16, tag="tp")
                for i, k in enumerate(tiles):
                    nc.tensor.transpose(
                        pt[:, i * 128:(i + 1) * 128],
                        sn_rows[:, k * 128:(k + 1) * 128], ident_bf[:])
                nc.vector.tensor_copy(snT[:, base: base + w], pt[:, :w])

            def _ptiles(idx):
                if idx < 0:
                    h = idx + 4
                    return range(h * 4, h * 4 + 4), True
                return range(idx * 8, idx * 8 + 8), False

            def emit_produce(p, t):
                lo, hi = PANELS[p]
                w = (hi - lo) * 128
                pg = pg_pool.tile([128, 1536], f32, tag="pg")
                lhsT = snT[:, t * 128:(t + 1) * 128]
                c0 = (t + lo) * 128
                for j in range(0, w, 512):
                    nc.tensor.matmul(pg[:, j:j + 512], lhsT,
                                     snT[:, c0 + j:c0 + j + 512],
                                     start=True, stop=True)
                ridx = O_ROW + t * NP + p
                if (p, t) in OFFLOAD:
                    q = scr_pool.tile([128, 1536], i32, tag="q")
                    # 512-col pieces so subtile deps release pg banks early
                    for j in range(0, w, 512):
                        nc.vector.tensor_scalar(
                            out=q[:, j:j + 512], in0=pg[:, j:j + 512],
                            scalar1=SCHRAUDOLPH_S, scalar2=SCHRAUDOLPH_B,
                            op0=OP.mult, op1=OP.add)
                    # fused row-sum of the bit-cast exps (DVE accumulator)
                    dmy = scr_pool.tile([128, 1536], f32, tag="dmyf")
                    nc.vector.tensor_scalar(
                        out=dmy[:, :w], in0=q[:, :w].bitcast(f32),
                        scalar1=1.0, scalar2=0.0, op0=OP.mult, op1=OP.add,
                        accum_out=outbuf[:, ridx:ridx + 1])
                    return (p, t, q[:].bitcast(f32), ones_f, identf, f32)
                ex = ex_pool.tile([128, 1536], bf16, tag="ex")
                nc.scalar.activation(out=ex[:, :w], in_=pg[:, :w],
                                     func=AF.Exp,
                                     accum_out=outbuf[:, ridx:ridx + 1])
                return (p, t, ex[:], ones_bf, ident_bf, bf16)

            def emit_consume(pend):
                p, t, exb, ones_t, id_t, dg_dt = pend
                lo, hi = PANELS[p]
                # per-block colsum mat-vecs into a per-unit PSUM tile
                # (disjoint columns, no PSUM accumulation across units),
                # then one DVE add into the SBUF accumulator
                b0 = max(lo, 1) - lo
                b1 = (min(hi - 1, 35) - lo) + 1
                csu = cs_pool.tile([128, 12], f32, tag="cs")
                for b in range(b0, b1):
                    nc.tensor.matmul(csu[:, b:b + 1],
                                     exb[:, b * 128:(b + 1) * 128],
                                     ones_t[:], start=True, stop=True)
                a0 = O_COL + t + lo - 1
                nc.vector.tensor_tensor(
                    out=outbuf[:, a0 + b0:a0 + b1],
                    in0=csu[:, b0:b1],
                    in1=outbuf[:, a0 + b0:a0 + b1], op=OP.add)
                for d in range(lo, hi):
                    if d in POS_DELTAS:
                        bsl = slice((d - lo) * 128, (d - lo + 1) * 128)
                        dsc = scr_pool.tile([128, 128], dg_dt,
                                            tag=f"diag{dg_dt}")
                        eidx = O_EP + t * 4 + d // 8 - 1
                        if (p, t) in ((2, 7), (2, 8)):
                            # tail units: single fused op, no Pool hop
                            nc.vector.scalar_tensor_tensor(
                                out=dsc, in0=exb[:, bsl], scalar=1.0,
                                in1=id_t[:], op0=OP.mult, op1=OP.mult,
                                accum_out=outbuf[:, eidx:eidx + 1])
                            continue
                        # identity-mask on Pool, fused row-sum on DVE (4x)
                        nc.gpsimd.tensor_tensor(
                            out=dsc, in0=exb[:, bsl], in1=id_t[:],
                            op=OP.mult)
                        dmyd = scr_pool.tile([128, 128], dg_dt,
                                             tag=f"dmyd{dg_dt}")
                        nc.vector.tensor_scalar(
                            out=dmyd, in0=dsc, scalar1=1.0, scalar2=0.0,
                            op0=OP.mult, op1=OP.add,
                            accum_out=outbuf[:, eidx:eidx + 1])

            def emit_d36(t):
                # rowsum-only block (Tb, Tb+36) via DVE Schraudolph; shares
                # the f32 cs PSUM slot (same dtype -- no aliased-slot races)
                pg = cs_pool.tile([128, 128], f32, tag="cs")
                lhsT = snT[:, t * 128:(t + 1) * 128]
                c0 = (t + 36) * 128
                nc.tensor.matmul(pg[:], lhsT, snT[:, c0:c0 + 128],
                                 start=True, stop=True)
                q = scr_pool.tile([128, 128], i32, tag="q36")
                nc.vector.tensor_scalar(
                    out=q[:], in0=pg[:],
                    scalar1=SCHRAUDOLPH_S, scalar2=SCHRAUDOLPH_B,
                    op0=OP.mult, op1=OP.add)
                dmy = scr_pool.tile([128, 128], f32, tag="dmy36")
                ridx = O_ROW + t * NP + 3
                nc.vector.tensor_scalar(
                    out=dmy[:], in0=q[:].bitcast(f32),
                    scalar1=1.0, scalar2=0.0, op0=OP.mult, op1=OP.add,
                    accum_out=outbuf[:, ridx:ridx + 1])

            # Software-pipelined emission: unit i's colsum mat-vecs and diag
            # extracts are emitted after unit i+1's matmuls+exp, so the PE
            # FIFO never stalls on a pending exp (head-of-line blocking).
            pending = []
            for item in _schedule():
                if item[0] == "norm":
                    tiles, fast = _ptiles(item[1])
                    prologue_norm(tiles, fast)
                elif item[0] == "trans":
                    tiles, _ = _ptiles(item[1])
                    prologue_trans(tiles)
                elif item[0] == "unit":
                    pending.append(emit_produce(item[1], item[2]))
                    if len(pending) > 2:
                        emit_consume(pending.pop(0))
                else:
                    emit_d36(item[1])
            for pend in pending:
                emit_consume(pend)

            nc.sync.dma_start(out=y_out[:], in_=outbuf[:])

    nc.finalize()
    return nc


def _get_nc():
    if "nc" not in _CACHE:
        _CACHE["nc"] = _build()
    return _CACHE["nc"]


def kernel(s: np.ndarray) -> np.ndarray:
    from concourse.bass_utils import run_bass_kernel_spmd

    s = np.ascontiguousarray(s, dtype=np.float32)
    assert s.shape == (K, D)
    nc = _get_nc()
    in_maps = []
    for c in range(NCORES):
        rolled = np.roll(s, -R * c, axis=0)[:NCHUNK * 1024]
        in_maps.append({"s": np.ascontiguousarray(rolled)})
    res = run_bass_kernel_spmd(nc, in_maps, core_ids=list(range(NCORES)))
    _CACHE["last_results"] = res

    rowtot = np.zeros(K, dtype=np.float64)
    pairs_i = []
    pairs_j = []
    pairs_v = []
    arange = np.arange(128)
    for c in range(NCORES):
        out = res.results[c]["out"].astype(np.float64)
        rowscr = out[:, O_ROW:O_ROW + RT * NP]
        colout = out[:, O_COL:O_COL + 43]
        epos = out[:, O_EP:O_EP + 36]
        base = R * c
        for t in range(RT):
            rows = base + t * 128 + arange
            rowtot[rows] += rowscr[:, t * NP:(t + 1) * NP].sum(axis=1)
        for cs in range(1, 44):
            cols = (base + cs * 128 + arange) % K
            rowtot[cols] += colout[:, cs - 1]
        for t in range(RT):
            for m in range(1, 5):
                i = base + t * 128 + arange
                j = (i + 1024 * m) % K
                pairs_i.append(i)
                pairs_j.append(j)
                pairs_v.append(epos[:, t * 4 + m - 1])

    pi = np.concatenate(pairs_i)
    pj = np.concatenate(pairs_j)
    pv = np.concatenate(pairs_v)

    possum = np.zeros(K, dtype=np.float64)
    np.add.at(possum, pi, pv)
    np.add.at(possum, pj, pv)
    negsum = rowtot - possum - E2

    terms = (np.log(pv + negsum[pi]) + np.log(pv + negsum[pj])
             - 2.0 * np.log(pv))
    loss = terms.sum() / KPOS
    return np.array(loss, dtype=np.float32)


# revision 67
# speedup vs baseline: 1.0045x; 1.0045x over previous
"""Bass/Trainium2 kernel for the DisentangleLoss (NT-Xent style contrastive loss).

Math (matches the reference):
    sn = s / max(||s||, eps) * sqrt(1/TEMP)      row-normalized embeddings
    sim = sn @ sn.T                              [K, K] similarity logits
    positives of row i: columns j != i with |i-j| = 1024*m   (8 per row)
    negatives of row i: everything else except the diagonal  (K-9 per row)
    loss = mean over (row, positive) of  logaddexp(p, lse(negatives)) - p

Device strategy (8 NeuronCores, SPMD) -- "circulant band" symmetric scheme:
  sim is symmetric, so only half the exp(sim) evaluations are needed.  Core c
  gets a row-rolled copy of s (np.roll by -1152*c) and computes, for its 9
  local row tiles Tb (128 rows each), the sim blocks (Tb, Tb+d) for band
  offsets d = 0..36.  Globally every unordered tile pair {A, B} is covered
  exactly once (d in 1..35), the diagonal tiles once (d=0), and the d=36 pairs
  exactly twice (so those skip their column sums).  Each core:
    * loads rows 0..6143 of its rolled s (tiles 0..44 are all it needs),
      normalizes (squares on Pool / fused sum-of-squares on DVE, rsqrt via
      ln/exp on ACT, per-tile scale+bf16-cast muls) and PE-transposes tiles
      0..44 into snT.  Emission order is tuned so the strict per-engine FIFOs
      never head-of-line block: norm phases fire as loads land, transpose
      phases a few units later, and each unit's colsum/diag consumption is
      deferred two units behind its matmul+exp production.
    * for each (row tile Tb, d-panel of 12): one bf16 matmul row [128, 1536]
      into PSUM, then exp via ACT (accum_out = row-sum partial; ex to SBUF
      bf16) or, for OFFLOAD units and the d=36 blocks, DVE Schraudolph
      bit-trick exp + fused f32 row-sum (balances the two PSUM readers).
    * column sums (the mirrored half of the symmetric matrix) via per-block
      "ones mat-vec": matmul(out=[128,1], lhsT=ex_block, rhs=ones) into a
      per-unit PSUM tile (PSUM accumulation across interleaved groups is NOT
      reliable on hw), added into an SBUF accumulator by one DVE op per unit.
    * positives live on the main diagonals of blocks d = 8,16,24,32;
      identity-mask multiply on Pool + fused row-sum accumulate on DVE
      (Pool supports no accumulators or scalar_tensor_tensor in the ISA).
    * self-similarity is exactly 2 after the sqrt(1/TEMP) scaling -> exp is
      the constant e^2, applied on the host.
  Each core outputs row-sum partials, column-sum partials and positive
  exponentials in one [128, 115] tensor; the host (O(K) numpy) assembles
  rowtot, per-row negsum and the final scalar loss.
"""

import math

import numpy as np

K = 9216
D = 128
BS = 1024
N = 9
TEMP = 0.5
NCORES = 8
R = K // NCORES          # 1152 rows per core
RT = R // 128            # 9 row tiles per core
NTILES = 45              # column tiles used per core (Tb + d <= 8 + 36)
NCHUNK = 6               # 1024-row load chunks (6144 rows >= 45*128)
KPOS = K * (N - 1)
E2 = math.exp(2.0)       # exp(self-similarity / TEMP)

# d-panels of 12 (ACT exp instructions are 1536 wide); d=36 handled separately
PANELS = [(0, 12), (12, 24), (24, 36)]
POS_DELTAS = (8, 16, 24, 32)
NP = len(PANELS) + 1     # rowscr columns per row tile (3 panels + d=36)

# Schraudolph bit-trick exp on DVE: exp(x) ~= bitcast_f32(int32(x*2^23/ln2+B));
# B fitted so the mean rel-err over the logit distribution is ~0 (max 3.3%
# per element, which averages out in the ~9k-term logsumexp).
SCHRAUDOLPH_S = 12102203.0
SCHRAUDOLPH_B = 1064951741.0
# (panel, Tb) units whose exp+rowsum runs on DVE instead of ACT
OFFLOAD = {(0, 7), (1, 2), (1, 5), (2, 1), (2, 4)}

# packed output layout
O_ROW = 0                # rowscr: NP per row tile        [0, 36)
O_COL = RT * NP          # colsums C=1..43                [36, 79)
O_EP = O_COL + 43        # exp(pos) per (Tb, m)           [79, 115)
O_W = O_EP + 36

_CACHE = {}


def _schedule():
    """Flattened main-loop schedule: list of ("unit", p, t) / ("d36", t) /
    ("chunk", ch).  All prologue chunks are emitted during panel 0 (each a
    little after its DMA load lands -- engine queues are strict FIFO, so a
    chunk emitted too early would head-of-line-block its engines); the d=36
    rowsum-only units are spread through panel 1."""
    norm_after = {(0, 1): 2, (0, 3): 3, (0, 5): 4, (0, 7): 5}
    trans_after = {(0, 4): 2, (0, 7): 3, (1, 1): 4, (1, 3): 5}
    sched = [("norm", -4), ("norm", -3), ("trans", -4), ("norm", -2),
             ("trans", -3), ("norm", -1), ("trans", -2), ("trans", -1)]
    for p, (lo, hi) in enumerate(PANELS):
        for t in range(RT):
            sched.append(("unit", p, t))
            if (p, t) in norm_after:
                sched.append(("norm", norm_after[(p, t)]))
            if (p, t) in trans_after:
                sched.append(("trans", trans_after[(p, t)]))
            # d36 units trail by 2 so their tiles (chunks 4-5) are ready
            if p == 1 and t >= 2:
                sched.append(("d36", t - 2))
            if p == 1 and t == 8:
                sched += [("d36", 7), ("d36", 8)]
    return sched


def _build():
    import concourse.bacc as bacc
    import concourse.tile as tile
    from concourse import mybir
    from concourse.masks import make_identity

    # Steer the ACT-table placement pass: every Exp/Ln in this kernel should
    # be served by the one set containing both ("natural_log_exp_and_others"),
    # otherwise the per-func first-match choice alternates tables and inserts
    # a ~2.7us ACT_TABLE_LOAD per switch.
    if not getattr(bacc, "_ant_act_tables_patched", False):
        _orig_get_tables = bacc.get_activation_tables

        def _patched_get_tables(arch):
            tables = dict(_orig_get_tables(arch))
            exp_ln = {mybir.ActivationFunctionType.Exp,
                      mybir.ActivationFunctionType.Ln}
            for name, funcs in tables.items():
                if name != "natural_log_exp_and_others" and \
                        exp_ln <= tables.get("natural_log_exp_and_others",
                                             set()):
                    tables[name] = funcs - exp_ln
            return tables

        bacc.get_activation_tables = _patched_get_tables
        bacc._ant_act_tables_patched = True

    f32 = mybir.dt.float32
    bf16 = mybir.dt.bfloat16
    i32 = mybir.dt.int32
    AF = mybir.ActivationFunctionType
    OP = mybir.AluOpType

    nc = bacc.Bacc("TRN2", target_bir_lowering=False, debug=False,
                   num_devices=NCORES)
    s_in = nc.dram_tensor("s", [NCHUNK * 1024, D], f32, kind="ExternalInput")
    y_out = nc.dram_tensor("out", [128, O_W], f32, kind="ExternalOutput")

    with tile.TileContext(nc) as tc:
        with (
            tc.tile_pool(name="big", bufs=1) as big,
            tc.tile_pool(name="small", bufs=1) as small,
            tc.tile_pool(name="ex", bufs=6) as ex_pool,
            tc.tile_pool(name="scr", bufs=4) as scr_pool,
            tc.tile_pool(name="pg", bufs=2, space="PSUM") as pg_pool,
            tc.tile_pool(name="tp", bufs=1, space="PSUM") as tp_pool,
            tc.tile_pool(name="cs", bufs=1, space="PSUM") as cs_pool,
        ):
            s_rows = big.tile([128, NCHUNK * 1024], f32)
            sn_rows = big.tile([128, NCHUNK * 1024], bf16)
            snT = big.tile([128, NTILES * 128], bf16)

            identf = small.tile([128, 128], f32)
            make_identity(nc, identf)
            ident_bf = small.tile([128, 128], bf16)
            nc.vector.tensor_copy(ident_bf[:], identf[:])
            ones_bf = small.tile([128, 1], bf16)
            nc.vector.memset(ones_bf, 1.0)
            ones_f = small.tile([128, 1], f32)
            nc.vector.memset(ones_f, 1.0)

            ss = small.tile([128, NCHUNK * 8], f32)   # per-row sum of squares
            lnss = small.tile([128, NCHUNK * 8], f32)
            sclr = small.tile([128, NCHUNK * 8], f32)
            scl = small.tile([128, NCHUNK * 8], f32)
            outbuf = small.tile([128, O_W], f32)      # packed outputs
            nc.vector.memset(outbuf[:, O_COL:O_COL + 43], 0.0)

            rt2 = math.sqrt(1.0 / TEMP)
            bias_t = small.tile([128, 1], f32)
            nc.vector.memset(bias_t, math.log(rt2))

            # tiny dummy exp so the ACT table loads while DMAs stream
            nc.scalar.activation(out=lnss[:, 0:1], in_=bias_t[:], func=AF.Exp)

            # ---- loads: chunks 0-1 as four 512-row half-loads (faster
            # startup; the first exp needs 12 tiles), chunks 2-5 whole.
            def emit_load(r0, r1, eng):
                src = s_in[r0:r1, :].rearrange("(t p) d -> p t d", p=128)
                dst = s_rows[:, r0:r1].rearrange("p (t d) -> p t d", d=128)
                eng.dma_start(out=dst, in_=src)

            emit_load(0, 512, nc.scalar)
            for h in range(1, 4):
                emit_load(h * 512, (h + 1) * 512, nc.sync)
            for ch in range(2, NCHUNK):
                emit_load(ch * 1024, (ch + 1) * 1024, nc.sync)

            def prologue_norm(tiles, fast):
                # fast=True puts the whole chain on DVE (kernel-start
                # critical path); otherwise squares and scale run on Pool
                # (its only ISA-legal roles) with row-sums+accum on DVE.
                tiles = [k for k in tiles if k < NTILES]
                if not tiles:
                    return
                w = len(tiles) * 128
                base = tiles[0] * 128
                if fast:
                    for k in tiles:
                        sl = slice(k * 128, (k + 1) * 128)
                        sc = scr_pool.tile([128, 128], f32, tag="ssq")
                        nc.vector.scalar_tensor_tensor(
                            out=sc, in0=s_rows[:, sl], scalar=1.0,
                            in1=s_rows[:, sl], op0=OP.mult, op1=OP.mult,
                            accum_out=ss[:, k:k + 1])
                else:
                    sq = scr_pool.tile([128, 1024], f32, tag="sq")
                    nc.gpsimd.tensor_tensor(
                        out=sq[:, :w], in0=s_rows[:, base:base + w],
                        in1=s_rows[:, base:base + w], op=OP.mult)
                    for i, k in enumerate(tiles):
                        dmy = scr_pool.tile([128, 128], f32, tag="ssqd")
                        nc.vector.tensor_scalar(
                            out=dmy, in0=sq[:, i * 128:(i + 1) * 128],
                            scalar1=1.0, scalar2=0.0, op0=OP.mult,
                            op1=OP.add, accum_out=ss[:, k:k + 1])
                gsl = slice(tiles[0], tiles[-1] + 1)
                nc.scalar.activation(out=lnss[:, gsl], in_=ss[:, gsl],
                                     func=AF.Ln)
                nc.scalar.activation(out=sclr[:, gsl], in_=lnss[:, gsl],
                                     func=AF.Exp, scale=-0.5, bias=bias_t[:])
                nc.vector.tensor_scalar_min(scl[:, gsl], sclr[:, gsl],
                                            rt2 * 1e8)
                # normalize+scale+cast to bf16 (per-tile scalar mul,
                # alternating Pool/DVE so neither serializes the chunk)
                for i, k in enumerate(tiles):
                    sl = slice(k * 128, (k + 1) * 128)
                    eng = nc.vector if (fast or i % 2) else nc.gpsimd
                    eng.tensor_scalar_mul(sn_rows[:, sl], s_rows[:, sl],
                                          scl[:, k:k + 1])

            def prologue_trans(tiles):
                # PE transpose into snT (via bf16 PSUM staging); emitted a
                # few units after the norm phase so the PE FIFO never waits
                # on the Pool-engine scale muls.
                tiles = [k for k in tiles if k < NTILES]
                if not tiles:
                    return
                w = len(tiles) * 128
                base = tiles[0] * 128
                pt = tp_pool.tile([128, 1024], bf16, tag="tp")
                for i, k in enumerate(tiles):
                    nc.tensor.transpose(
                        pt[:, i * 128:(i + 1) * 128],
                        sn_rows[:, k * 128:(k + 1) * 128], ident_bf[:])
                nc.vector.tensor_copy(snT[:, base: base + w], pt[:, :w])

            def _ptiles(idx):
                if idx < 0:
                    h = idx + 4
                    return range(h * 4, h * 4 + 4), True
                return range(idx * 8, idx * 8 + 8), False

            def emit_produce(p, t):
                lo, hi = PANELS[p]
                w = (hi - lo) * 128
                pg = pg_pool.tile([128, 1536], f32, tag="pg")
                lhsT = snT[:, t * 128:(t + 1) * 128]
                c0 = (t + lo) * 128
                for j in range(0, w, 512):
                    nc.tensor.matmul(pg[:, j:j + 512], lhsT,
                                     snT[:, c0 + j:c0 + j + 512],
                                     start=True, stop=True)
                ridx = O_ROW + t * NP + p
                if (p, t) in OFFLOAD:
                    q = scr_pool.tile([128, 1536], i32, tag="q")
                    # 512-col pieces so subtile deps release pg banks early
                    for j in range(0, w, 512):
                        nc.vector.tensor_scalar(
                            out=q[:, j:j + 512], in0=pg[:, j:j + 512],
                            scalar1=SCHRAUDOLPH_S, scalar2=SCHRAUDOLPH_B,
                            op0=OP.mult, op1=OP.add)
                    # fused row-sum of the bit-cast exps is deferred to
                    # the consume phase (rowscr is only read at the end;
                    # this keeps DVE free in the offload unit's window)
                    return (p, t, q[:].bitcast(f32), ones_f, identf, f32,
                            (q, w, ridx))
                ex = ex_pool.tile([128, 1536], bf16, tag="ex")
                nc.scalar.activation(out=ex[:, :w], in_=pg[:, :w],
                                     func=AF.Exp,
                                     accum_out=outbuf[:, ridx:ridx + 1])
                return (p, t, ex[:], ones_bf, ident_bf, bf16, None)

            def emit_consume(pend):
                p, t, exb, ones_t, id_t, dg_dt, qrow = pend
                lo, hi = PANELS[p]
                if qrow is not None:
                    q, w, ridx = qrow
                    dmy = scr_pool.tile([128, 1536], f32, tag="dmyf")
                    nc.vector.tensor_scalar(
                        out=dmy[:, :w], in0=q[:, :w].bitcast(f32),
                        scalar1=1.0, scalar2=0.0, op0=OP.mult, op1=OP.add,
                        accum_out=outbuf[:, ridx:ridx + 1])
                # per-block colsum mat-vecs into a per-unit PSUM tile
                # (disjoint columns, no PSUM accumulation across units),
                # then one DVE add into the SBUF accumulator
                b0 = max(lo, 1) - lo
                b1 = (min(hi - 1, 35) - lo) + 1
                csu = cs_pool.tile([128, 12], f32, tag="cs")
                for b in range(b0, b1):
                    nc.tensor.matmul(csu[:, b:b + 1],
                                     exb[:, b * 128:(b + 1) * 128],
                                     ones_t[:], start=True, stop=True)
                a0 = O_COL + t + lo - 1
                nc.vector.tensor_tensor(
                    out=outbuf[:, a0 + b0:a0 + b1],
                    in0=csu[:, b0:b1],
                    in1=outbuf[:, a0 + b0:a0 + b1], op=OP.add)
                for d in range(lo, hi):
                    if d in POS_DELTAS:
                        bsl = slice((d - lo) * 128, (d - lo + 1) * 128)
                        dsc = scr_pool.tile([128, 128], dg_dt,
                                            tag=f"diag{dg_dt}")
                        eidx = O_EP + t * 4 + d // 8 - 1
                        if (p, t) in ((2, 7), (2, 8)):
                            # tail units: single fused op, no Pool hop
                            nc.vector.scalar_tensor_tensor(
                                out=dsc, in0=exb[:, bsl], scalar=1.0,
                                in1=id_t[:], op0=OP.mult, op1=OP.mult,
                                accum_out=outbuf[:, eidx:eidx + 1])
                            continue
                        # identity-mask on Pool, fused row-sum on DVE (4x)
                        nc.gpsimd.tensor_tensor(
                            out=dsc, in0=exb[:, bsl], in1=id_t[:],
                            op=OP.mult)
                        dmyd = scr_pool.tile([128, 128], dg_dt,
                                             tag=f"dmyd{dg_dt}")
                        nc.vector.tensor_scalar(
                            out=dmyd, in0=dsc, scalar1=1.0, scalar2=0.0,
                            op0=OP.mult, op1=OP.add,
                            accum_out=outbuf[:, eidx:eidx + 1])

            def emit_d36(t):
                # rowsum-only block (Tb, Tb+36) via DVE Schraudolph; shares
                # the f32 cs PSUM slot (same dtype -- no aliased-slot races)
                pg = cs_pool.tile([128, 128], f32, tag="cs")
                lhsT = snT[:, t * 128:(t + 1) * 128]
                c0 = (t + 36) * 128
                nc.tensor.matmul(pg[:], lhsT, snT[:, c0:c0 + 128],
                                 start=True, stop=True)
                q = scr_pool.tile([128, 128], i32, tag="q36")
                nc.vector.tensor_scalar(
                    out=q[:], in0=pg[:],
                    scalar1=SCHRAUDOLPH_S, scalar2=SCHRAUDOLPH_B,
                    op0=OP.mult, op1=OP.add)
                dmy = scr_pool.tile([128, 128], f32, tag="dmy36")
                ridx = O_ROW + t * NP + 3
                nc.vector.tensor_scalar(
                    out=dmy[:], in0=q[:].bitcast(f32),
                    scalar1=1.0, scalar2=0.0, op0=OP.mult, op1=OP.add,
                    accum_out=outbuf[:, ridx:ridx + 1])

            # Software-pipelined emission: unit i's colsum mat-vecs and diag
            # extracts are emitted after unit i+1's matmuls+exp, so the PE
            # FIFO never stalls on a pending exp (head-of-line blocking).
            pending = []
            for item in _schedule():
                if item[0] == "norm":
                    tiles, fast = _ptiles(item[1])
                    prologue_norm(tiles, fast)
                elif item[0] == "trans":
                    tiles, _ = _ptiles(item[1])
                    prologue_trans(tiles)
                elif item[0] == "unit":
                    pending.append(emit_produce(item[1], item[2]))
                    if len(pending) > 2:
                        emit_consume(pending.pop(0))
                else:
                    emit_d36(item[1])
            for pend in pending:
                emit_consume(pend)

            nc.sync.dma_start(out=y_out[:], in_=outbuf[:])

    nc.finalize()
    return nc


def _get_nc():
    if "nc" not in _CACHE:
        _CACHE["nc"] = _build()
    return _CACHE["nc"]


def kernel(s: np.ndarray) -> np.ndarray:
    from concourse.bass_utils import run_bass_kernel_spmd

    s = np.ascontiguousarray(s, dtype=np.float32)
    assert s.shape == (K, D)
    nc = _get_nc()
    in_maps = []
    for c in range(NCORES):
        rolled = np.roll(s, -R * c, axis=0)[:NCHUNK * 1024]
        in_maps.append({"s": np.ascontiguousarray(rolled)})
    res = run_bass_kernel_spmd(nc, in_maps, core_ids=list(range(NCORES)))
    _CACHE["last_results"] = res

    rowtot = np.zeros(K, dtype=np.float64)
    pairs_i = []
    pairs_j = []
    pairs_v = []
    arange = np.arange(128)
    for c in range(NCORES):
        out = res.results[c]["out"].astype(np.float64)
        rowscr = out[:, O_ROW:O_ROW + RT * NP]
        colout = out[:, O_COL:O_COL + 43]
        epos = out[:, O_EP:O_EP + 36]
        base = R * c
        for t in range(RT):
            rows = base + t * 128 + arange
            rowtot[rows] += rowscr[:, t * NP:(t + 1) * NP].sum(axis=1)
        for cs in range(1, 44):
            cols = (base + cs * 128 + arange) % K
            rowtot[cols] += colout[:, cs - 1]
        for t in range(RT):
            for m in range(1, 5):
                i = base + t * 128 + arange
                j = (i + 1024 * m) % K
                pairs_i.append(i)
                pairs_j.append(j)
                pairs_v.append(epos[:, t * 4 + m - 1])

    pi = np.concatenate(pairs_i)
    pj = np.concatenate(pairs_j)
    pv = np.concatenate(pairs_v)

    possum = np.zeros(K, dtype=np.float64)
    np.add.at(possum, pi, pv)
    np.add.at(possum, pj, pv)
    negsum = rowtot - possum - E2

    terms = (np.log(pv + negsum[pi]) + np.log(pv + negsum[pj])
             - 2.0 * np.log(pv))
    loss = terms.sum() / KPOS
    return np.array(loss, dtype=np.float32)
